# revision 6
# baseline (speedup 1.0000x reference)
"""AdderNet on 8 TRN2 cores — low-instruction-count architecture.

Per conv: activations live in DRAM as img [ci, B, H, W] f16. Build a
replicated per-partition window buffer Yf via broadcast DMAs (partition
p = co*lhn+lh holds the rows its output slice needs), then 9 in-place
tensor_tensor subtracts (one per kernel offset s) into D[128, s, l', ci],
tensor_reduce(|.|, negate) over ci, tensor_reduce over s -> R = -sum|x-w|.
BN stats via accum_out + AllReduce; evac with ACT bias/scale.
"""
import numpy as np

CORES = 8
BL = 4
GB = 32
EPS = 1e-5

# stage geometries keyed by out-geom
GEOM = {
    "X": dict(cn=16, lhn=8, lp=512, H=32),
    "Y": dict(cn=32, lhn=4, lp=256, H=16),
    "Z": dict(cn=64, lhn=2, lp=128, H=8),
}


def make_sched():
    cv = []

    def add(name, gin, gout, ci, k, st, wsrc, evac, idn=None, yin=None, yout=None):
        cv.append(dict(name=name, gin=gin, gout=gout, ci=ci, k=k, st=st,
                       wsrc=wsrc, evac=evac, idn=idn, yin=yin, yout=yout))

    add("stem", "S", "X", 3, 3, 1, ("conv1_w",), "relu", yout=0)
    rot = [(0, 1, 2), (2, 0, 1), (1, 2, 0)]
    for b in range(3):
        i, m, o = rot[b]
        add(f"l1b{b}c1", "X", "X", 16, 3, 1, ("l1_w", 2 * b), "relu", yin=i, yout=m)
        add(f"l1b{b}c2", "X", "X", 16, 3, 1, ("l1_w", 2 * b + 1), "res", idn=i,
            yin=m, yout=o)
    add("l2tc1", "TX", "Y", 16, 3, 2, ("l2_w0",), "relu", yin=1, yout=0)
    add("l2td", "TX", "Y", 16, 1, 2, ("l2_down",), "iden", yin=1, yout=3)
    add("l2tc2", "Y", "Y", 32, 3, 1, ("l2_ws", 0), "res", idn=3, yin=0, yout=1)
    rot2 = [(1, 2, 0), (0, 2, 1)]
    for b in range(2):
        i, m, o = rot2[b]
        add(f"l2b{b}c1", "Y", "Y", 32, 3, 1, ("l2_ws", 1 + 2 * b), "relu", yin=i, yout=m)
        add(f"l2b{b}c2", "Y", "Y", 32, 3, 1, ("l2_ws", 2 + 2 * b), "res", idn=i,
            yin=m, yout=o)
    add("l3tc1", "TY", "Z", 32, 3, 2, ("l3_w0",), "relu", yin=1, yout=0)
    add("l3td", "TY", "Z", 32, 1, 2, ("l3_down",), "iden", yin=1, yout=3)
    add("l3tc2", "Z", "Z", 64, 3, 1, ("l3_ws", 0), "res", idn=3, yin=0, yout=1)
    for b in range(2):
        i, m, o = rot2[b]
        add(f"l3b{b}c1", "Z", "Z", 64, 3, 1, ("l3_ws", 1 + 2 * b), "relu", yin=i, yout=m)
        add(f"l3b{b}c2", "Z", "Z", 64, 3, 1, ("l3_ws", 2 + 2 * b), "res", idn=i,
            yin=m, yout=o)
    return cv


SCHED = make_sched()


def woffsets():
    offs = {}
    o = 0
    for m in SCHED:
        offs[m["name"]] = o
        o += m["k"] * m["k"] * m["ci"]
    return offs, o


WOFF, WTOT = woffsets()


def get_w(inputs, wsrc):
    a = inputs[wsrc[0]]
    if len(wsrc) > 1:
        a = a[wsrc[1]]
    return a


def pack_host(inputs):
    wbig = np.zeros((128, WTOT), np.float16)
    for meta in SCHED:
        w = get_w(inputs, meta["wsrc"])  # [co, ci, k, k]
        g = GEOM[meta["gout"]]
        cn, lhn = g["cn"], g["lhn"]
        k, ci = meta["k"], meta["ci"]
        off = WOFF[meta["name"]]
        wf = w.reshape(cn, ci, k * k).transpose(0, 2, 1).reshape(cn, k * k * ci)
        for p in range(128):
            wbig[p, off:off + k * k * ci] = wf[p // lhn]
    # stem input: [128, 3, 18, 34] per core
    x = inputs["x"].astype(np.float16)  # [32, 3, 32, 32]
    xs = x.reshape(CORES, BL, 3, 32, 32)
    xstem = np.zeros((CORES, 128, 3, 18, 34), np.float16)
    xpad = np.zeros((CORES, BL, 3, 34, 34), np.float16)
    xpad[:, :, :, 1:33, 1:33] = xs
    for p in range(128):
        lh = p % 8
        b, rh = divmod(lh, 2)
        xstem[:, p] = xpad[:, b, :, rh * 16:rh * 16 + 18, :]
    # fc lhsT [128, 20] f32: cols lh*10..+10 = fcw[cls, co]/64 on partitions with
    # p % lhn == lh... p=(co,lh): lh = p % 2, co = p // 2
    fcl = np.zeros((128, 20), np.float32)
    fcw = inputs["fc_w"][:, :, 0, 0]  # [10, 64]
    for p in range(128):
        co, lh = p // 2, p % 2
        fcl[p, lh * 10:(lh + 1) * 10] = fcw[:, co] / 64.0
    sel = np.zeros((64, 384), np.float32)
    for j, lhn in enumerate((8, 4, 2)):
        for p in range(128):
            sel[p // lhn, j * 128 + p] = 1.0
    return wbig, xstem, fcl, sel


_CACHE = {}


def build(debug=False, reps=1):
    from concourse import bacc, mybir, tile

    F16, F32 = mybir.dt.float16, mybir.dt.float32
    A = mybir.AluOpType
    AF = mybir.ActivationFunctionType
    AX = mybir.AxisListType

    nc = bacc.Bacc("TRN2", target_bir_lowering=False, debug=False,
                   num_devices=CORES)
    wbig_d = nc.dram_tensor("wbig", [128, WTOT], F16, kind="ExternalInput")
    xstem_d = nc.dram_tensor("xstem", [128, 3 * 18 * 34], F16, kind="ExternalInput")
    fcl_d = nc.dram_tensor("fcl", [128, 20], F32, kind="ExternalInput")
    sel_d = nc.dram_tensor("sel", [64, 384], F32, kind="ExternalInput")
    out_d = nc.dram_tensor("out", [10, BL], F32, kind="ExternalOutput")
    dbg_d = {}
    if debug:
        for meta in SCHED:
            g = GEOM[meta["gout"]]
            dbg_d[meta["name"]] = nc.dram_tensor(
                f'dbg_{meta["name"]}', [128, g["lp"]], F16, kind="ExternalOutput")

    with tile.TileContext(nc) as tc:
        import contextlib
        with contextlib.ExitStack() as ctx:
            pp = ctx.enter_context(tc.tile_pool(name="persist", bufs=1))
            psp = ctx.enter_context(tc.tile_pool(name="psum", bufs=1, space="PSUM"))
            drp = ctx.enter_context(tc.tile_pool(name="dram", bufs=1, space="DRAM"))

            wc = pp.tile([128, 576], F16, tag="wc")
            fcl = pp.tile([128, 20], F32, tag="fcl")
            sel = pp.tile([64, 384], F32, tag="sel")
            nc.sync.dma_start(fcl[:], fcl_d[:])
            nc.sync.dma_start(sel[:], sel_d[:])
            epst = pp.tile([64, 1], F32, tag="epst")
            nc.vector.memset(epst[:], EPS)

            D = pp.tile([128, 73728], F16, tag="D")
            nc.vector.memset(D[:, 0:1024], 0.0)
            Yf = pp.tile([128, 20736], F16, tag="Yf")
            o1 = pp.tile([128, 4608], F16, tag="o1")
            R = pp.tile([128, 512], F32, tag="R")
            junk = D[:, 0:512]
            tres = D[:, 512:1024]
            yn = [pp.tile([128, 512], F16, tag=f"yn{i}", name=f"yn{i}")
                  for i in range(4)]
            stats = pp.tile([128, 2], F32, tag="stats")
            gst = pp.tile([64, 8, 2], F32, tag="gst")
            g2 = pp.tile([64, 2], F32, tag="g2")
            bnm = pp.tile([64, 2], F32, tag="bnm")
            rr = pp.tile([64, 2], F32, tag="rr")
            rrb = pp.tile([128, 2], F32, tag="rrb")

            imgX = drp.tile([16, 4, 32, 34], F16, tag="imgX")
            imgY = drp.tile([32, 4, 16, 18], F16, tag="imgY")
            imgZ = drp.tile([64, 4, 8, 10], F16, tag="imgZ")
            sin = drp.tile([128, 2], F32, tag="sin")
            sout = drp.tile([128, 2], F32, tag="sout")
            sinf = drp.tile([10, 2], F32, tag="sinf")
            soutf = drp.tile([10, 2], F32, tag="soutf")
            IMG = {"X": imgX, "Y": imgY, "Z": imgZ}
            nc.sync.dma_start(
                imgX.rearrange("c b (rh r) w -> (c b rh) (r w)", rh=2),
                D[:, 0:544])
            nc.sync.dma_start(
                imgY.rearrange("c b h w -> (c b) (h w)"), D[:, 0:288])
            nc.sync.dma_start(
                imgZ.rearrange("c (bh b2) h w -> (c bh) (b2 h w)", b2=2),
                D[:, 0:160])


            # Yf views per input-geometry
            def yf_view(gin):
                if gin == "S":
                    return Yf[:, :3 * 18 * 34].rearrange(
                        "p (c h w) -> p c h w", c=3, h=18)
                if gin == "X":
                    return Yf[:, :16 * 18 * 34].rearrange(
                        "p (c h w) -> p c h w", c=16, h=18)
                if gin == "Y":
                    return Yf[:, :32 * 18 * 18].rearrange(
                        "p (c h w) -> p c h w", c=32, h=18)
                if gin == "Z":
                    return Yf[:, :64 * 20 * 10].rearrange(
                        "p (c h w) -> p c h w", c=64, h=20)
                if gin == "TX":
                    return Yf[:, :16 * 34 * 34].rearrange(
                        "p (c h w) -> p c h w", c=16, h=34)
                if gin == "TY":
                    return Yf[:, :32 * 36 * 18].rearrange(
                        "p (c h w) -> p c h w", c=32, h=36)

            def build_yf(meta):
                """Regather input img -> Yf (replicated windows)."""
                gin = meta["gin"]
                go = GEOM[meta["gout"]]
                yv = yf_view(gin)
                if gin == "S":
                    nc.sync.dma_start(Yf[:, :3 * 18 * 34], xstem_d[:])
                    return
                if gin == "X":  # 18-row slices per lh, full 34-wide rows
                    for lh in range(8):
                        b, rh = divmod(lh, 2)
                        r0 = rh * 16 - 1
                        r1 = rh * 16 + 17
                        c0, c1 = max(r0, 0), min(r1, 32)
                        dr0 = c0 - r0
                        src = imgX[None, :, b, c0:c1, :].broadcast_to(
                            [16, 16, c1 - c0, 34]).rearrange(
                            "q c r w -> q c (r w)")
                        dst = yv[lh::8, :, dr0:dr0 + c1 - c0, :].rearrange(
                            "p c r w -> p c (r w)")
                        nc.sync.dma_start(dst, src)
                elif gin == "Y":
                    for lh in range(4):
                        src = imgY[None, :, lh, :, :].broadcast_to(
                            [32, 32, 16, 18]).rearrange("q c r w -> q c (r w)")
                        dst = yv[lh::4, :, 1:17, :].rearrange(
                            "p c r w -> p c (r w)")
                        nc.sync.dma_start(dst, src)
                elif gin == "Z":
                    for lh in range(2):
                        for b2 in range(2):
                            src = imgZ[None, :, 2 * lh + b2, :, :].broadcast_to(
                                [64, 64, 8, 10]).rearrange(
                                "q c r w -> q c (r w)")
                            dst = yv[lh::2, :, b2 * 10 + 1:b2 * 10 + 9,
                                     :].rearrange("p c r w -> p c (r w)")
                            nc.sync.dma_start(dst, src)
                elif gin == "TX":
                    for lh in range(4):
                        src = imgX[None, :, lh, :, :].broadcast_to(
                            [32, 16, 32, 34]).rearrange("q c r w -> q c (r w)")
                        dst = yv[lh::4, :, 1:33, :].rearrange(
                            "p c r w -> p c (r w)")
                        nc.sync.dma_start(dst, src)
                elif gin == "TY":
                    for lh in range(2):
                        for b2 in range(2):
                            src = imgY[None, :, 2 * lh + b2, :, :].broadcast_to(
                                [64, 32, 16, 18]).rearrange(
                                "q c r w -> q c (r w)")
                            dst = yv[lh::2, :, b2 * 18 + 1:b2 * 18 + 17,
                                     :].rearrange("p c r w -> p c (r w)")
                            nc.sync.dma_start(dst, src)

            def conv_D(meta):
                """tt subtracts + treds -> R[:, :lp] = -sum|x-w| (or conv)."""
                gin, gout = meta["gin"], meta["gout"]
                go = GEOM[gout]
                ci, k, st = meta["ci"], meta["k"], meta["st"]
                off = WOFF[meta["name"]]
                nc.sync.dma_start(wc[:, :k * k * ci],
                                  wbig_d[:, off:off + k * k * ci])
                yv = yf_view(gin)
                is_stem = meta["name"] == "stem"
                op = A.mult if is_stem else A.subtract
                kk = k * k
                ABS = dict(apply_absolute_value=not is_stem,
                           negate=not is_stem)
                if gout in ("X", "Y"):
                    H = go["H"] if gout == "Y" else 16
                    W = 32 if gout == "X" else 16
                    dv = D[:, :kk * H * W * ci].rearrange(
                        "p (s r w c) -> p s r w c", s=kk, r=H, w=W)
                    for s in range(kk):
                        kh, kw = divmod(s, k)
                        if k == 1:
                            sv = yv[:, :, 1:33:2, 1:33:2]
                        elif st == 2:
                            sv = yv[:, :, kh:kh + 2 * H:2, kw:kw + 2 * W:2]
                        else:
                            sv = yv[:, :, kh:kh + H, kw:kw + W]
                        wv = wc[:, s * ci:(s + 1) * ci][
                            :, None, None, :].broadcast_to([128, H, W, ci])
                        nc.vector.tensor_tensor(dv[:, s], sv.transpose(
                            [0, 2, 3, 1]), wv, op)
                    dm = D[:, :kk * H * W * ci].rearrange(
                        "p (sr w c) -> p sr w c", w=W, c=ci)
                    ov = o1[:, :kk * H * W].rearrange(
                        "p (sr w) -> p sr w", w=W)
                    nch = 2 if kk * H * W * ci > 65536 else 1
                    step = kk * H // nch
                    for c in range(nch):
                        with nc.allow_low_precision(reason="f32 accum f16 out"):
                            nc.vector.tensor_reduce(
                                ov[:, c * step:(c + 1) * step, :],
                                dm[:, c * step:(c + 1) * step, :, :],
                                AX.X, A.add, **ABS)
                    o3 = o1[:, :kk * H * W].rearrange(
                        "p (s r w) -> p s r w", s=kk, r=H)
                    rv = R[:, :H * W].rearrange("p (r w) -> p r w", r=H)
                    if kk == 1:
                        nc.vector.tensor_copy(rv[:], o3[:, 0])
                    else:
                        nc.vector.tensor_reduce(rv[:], o3.transpose(
                            [0, 2, 3, 1]), AX.X, A.add)
                else:  # gout Z: stacked pair [ci, 20, 10]
                    # out-row grid: stride1 -> 18 rows (im0 0..7, junk 8,9,
                    # im1 10..17); stride2 -> 17 rows (junk row 8 only)
                    G = 18 if st == 1 else 17
                    rb = 10 if st == 1 else 9
                    sgroups = [(0, 5), (5, kk)] if kk > 1 else [(0, 1)]
                    rv = R[:, :128].rearrange("p (b r w) -> p b r w", b=2, r=8)
                    for (s0, s1) in sgroups:
                        ns = s1 - s0
                        dv = D[:, :ns * G * 8 * ci].rearrange(
                            "p (s r w c) -> p s r w c", s=ns, r=G, w=8)
                        for s in range(s0, s1):
                            kh, kw = divmod(s, k)
                            if k == 1:
                                sv = yv[:, :, 1:1 + 2 * G:2, 1:17:2]
                            elif st == 2:
                                sv = yv[:, :, kh:kh + 2 * G:2, kw:kw + 16:2]
                            else:
                                sv = yv[:, :, kh:kh + G, kw:kw + 8]
                            wv = wc[:, s * ci:(s + 1) * ci][
                                :, None, None, :].broadcast_to([128, G, 8, ci])
                            nc.vector.tensor_tensor(
                                dv[:, s - s0], sv.transpose([0, 2, 3, 1]),
                                wv, A.subtract)
                        dm = D[:, :ns * G * 8 * ci].rearrange(
                            "p (sr w c) -> p sr w c", w=8, c=ci)
                        ov = o1[:, s0 * G * 8:s1 * G * 8].rearrange(
                            "p (s r w) -> p s r w", s=ns, r=G)
                        with nc.allow_low_precision(reason="f32 accum f16 out"):
                            nc.vector.tensor_reduce(
                                ov[:].rearrange("p s r w -> p (s r) w"),
                                dm[:], AX.X, A.add, **ABS)
                    o3 = o1[:, :kk * G * 8].rearrange(
                        "p (s r w) -> p s r w", s=kk, r=G)
                    for b2 in range(2):
                        r0 = b2 * rb
                        if kk == 1:
                            nc.vector.tensor_copy(
                                rv[:, b2], o3[:, 0, r0:r0 + 8, :])
                        else:
                            nc.vector.tensor_reduce(
                                rv[:, b2],
                                o3[:, :, r0:r0 + 8, :].transpose([0, 2, 3, 1]),
                                AX.X, A.add)

            def bn_evac(meta):
                g = GEOM[meta["gout"]]
                cn, lhn, lp = g["cn"], g["lhn"], g["lp"]
                n = GB * g["H"] * g["H"]
                # stats
                nc.vector.tensor_scalar(junk[:, :lp], R[:, :lp], 0.0, None,
                                        A.add, A.add, accum_out=stats[:, 0:1])
                nc.scalar.activation(junk[:, :lp], R[:, :lp], AF.Square,
                                     accum_out=stats[:, 1:2])
                nc.sync.dma_start(sin[:], stats[:])
                nc.gpsimd.collective_compute(
                    "AllReduce", A.add, replica_groups=[list(range(CORES))],
                    ins=[sin.opt()], outs=[sout.opt()])
                nc.sync.dma_start(
                    gst[:cn, :lhn, :],
                    sout[:].rearrange("(c l) s -> c l s", c=cn))
                nc.vector.tensor_reduce(g2[:cn], gst[:cn, :lhn, :].transpose(
                    [0, 2, 1]), AX.X, A.add)
                # bn math on [cn]
                nc.vector.tensor_scalar(bnm[:cn, 0:1], g2[:cn, 0:1], 1.0 / n,
                                        None, A.mult)
                nc.vector.tensor_tensor(bnm[:cn, 1:2], bnm[:cn, 0:1],
                                        bnm[:cn, 0:1], A.mult)
                nc.vector.tensor_scalar(rr[:cn, 0:1], g2[:cn, 1:2], 1.0 / n,
                                        bnm[:cn, 1:2], A.mult, A.subtract)
                nc.scalar.activation(bnm[:cn, 1:2], rr[:cn, 0:1], AF.Sqrt,
                                     bias=epst[:cn])
                nc.vector.reciprocal(rr[:cn, 0:1], bnm[:cn, 1:2])
                nc.vector.tensor_scalar(rr[:cn, 1:2], bnm[:cn, 0:1], -1.0,
                                        rr[:cn, 0:1], A.mult, A.mult)
                gsl = {8: 0, 4: 1, 2: 2}[lhn]
                rrp = psp.tile([128, 2], F32, tag="rrp", name="rrp")
                nc.tensor.matmul(rrp[:, :],
                                 sel[:cn, gsl * 128:(gsl + 1) * 128],
                                 rr[:cn, :], start=True, stop=True)
                nc.vector.tensor_copy(rrb[:], rrp[:])
                # evac
                ynout = yn[meta["yout"]]
                if meta["evac"] == "relu":
                    nc.scalar.activation(ynout[:, :lp], R[:, :lp], AF.Relu,
                                         bias=rrb[:, 1:2], scale=rrb[:, 0:1])
                elif meta["evac"] == "iden":
                    nc.scalar.activation(ynout[:, :lp], R[:, :lp], AF.Identity,
                                         bias=rrb[:, 1:2], scale=rrb[:, 0:1])
                else:  # res
                    idt = yn[meta["idn"]]
                    nc.vector.scalar_tensor_tensor(
                        tres[:, :lp], R[:, :lp], rrb[:, 0:1], idt[:, :lp],
                        A.mult, A.add)
                    nc.scalar.activation(ynout[:, :lp], tres[:, :lp], AF.Relu,
                                         bias=rrb[:, 1:2])
                # write img (skip for downsample evac "iden")
                if meta["evac"] != "iden":
                    img = IMG[meta["gout"]]
                    if meta["gout"] == "X":
                        dst = img.rearrange("c b (rh r) w -> (c b rh) r w",
                                            rh=2)[:, :, 1:33]
                    elif meta["gout"] == "Y":
                        dst = img.rearrange("c b h w -> (c b) h w")[:, :, 1:17]
                    else:
                        dst = img.rearrange("c (bh b2) h w -> (c bh) (b2 h) w",
                                            b2=2)[:, :, 1:9]
                    nc.sync.dma_start(dst, ynout[:, :lp])
                if debug:
                    nc.sync.dma_start(dbg_d[meta["name"]][:], ynout[:, :lp])

            # zero Yf at geometry switches
            prev_gin = None
            for _rep in range(reps):
                for meta in SCHED:
                    if meta["gin"] != prev_gin:
                        nc.vector.memset(Yf[:], 0.0)
                        prev_gin = meta["gin"]
                        build_yf(meta)
                    elif meta["gin"] not in ("TX", "TY"):
                        build_yf(meta)
                    with nc.named_scope(meta["name"]):
                        conv_D(meta)
                        bn_evac(meta)

                # ---- fc ----
                with nc.named_scope("fc"):
                    ynf = yn[SCHED[-1]["yout"]]
                    pool = pp.tile([128, 2], F32, tag="pool")
                    yv = ynf[:, :128].rearrange("p (b f) -> p b f", b=2)
                    for b2 in range(2):
                        nc.scalar.activation(junk[:, :64], yv[:, b2], AF.Identity,
                                             accum_out=pool[:, b2:b2 + 1])
                    psfc = psp.tile([10, 4], F32, tag="psfc")
                    for lh in range(2):
                        nc.tensor.matmul(psfc[:, lh * 2:lh * 2 + 2],
                                         fcl[:, lh * 10:(lh + 1) * 10],
                                         pool[:], start=True, stop=True)
                    stf = pp.tile([10, 2], F32, tag="stf")
                    jf = pp.tile([10, 4], F16, tag="jf")
                    nc.scalar.activation(jf[:], psfc[:], AF.Identity,
                                         accum_out=stf[:, 0:1])
                    nc.scalar.activation(jf[:], psfc[:], AF.Square,
                                         accum_out=stf[:, 1:2])
                    nc.sync.dma_start(sinf[:], stf[:])
                    nc.gpsimd.collective_compute(
                        "AllReduce", A.add, replica_groups=[list(range(CORES))],
                        ins=[sinf.opt()], outs=[soutf.opt()])
                    gstf = pp.tile([10, 2], F32, tag="gstf")
                    nc.sync.dma_start(gstf[:], soutf[:])
                    nc.vector.tensor_scalar(bnm[:10, 0:1], gstf[:, 0:1],
                                            1.0 / GB, None, A.mult)
                    nc.vector.tensor_tensor(bnm[:10, 1:2], bnm[:10, 0:1],
                                            bnm[:10, 0:1], A.mult)
                    nc.vector.tensor_scalar(rr[:10, 0:1], gstf[:, 1:2],
                                            1.0 / GB, bnm[:10, 1:2], A.mult,
                                            A.subtract)
                    nc.scalar.activation(bnm[:10, 1:2], rr[:10, 0:1], AF.Sqrt,
                                         bias=epst[:10])
                    nc.vector.reciprocal(rr[:10, 0:1], bnm[:10, 1:2])
                    nc.vector.tensor_scalar(rr[:10, 1:2], bnm[:10, 0:1], -1.0,
                                            rr[:10, 0:1], A.mult, A.mult)
                    osb = pp.tile([10, 4], F32, tag="osb")
                    nc.scalar.activation(osb[:], psfc[:], AF.Identity,
                                         bias=rr[:10, 1:2], scale=rr[:10, 0:1])
                    nc.sync.dma_start(out_d[:], osb[:])
                prev_gin = None if reps > 1 else prev_gin

    nc.compile()
    return nc


def get_nc(debug=False, reps=1):
    key = f"nc{debug}_{reps}"
    if key not in _CACHE:
        _CACHE[key] = build(debug, reps)
    return _CACHE[key]


def kernel(**inputs):
    from concourse.bass_utils import run_bass_kernel_spmd

    wbig, xstem, fcl, sel = pack_host(inputs)
    nc = get_nc()
    in_maps = [{"wbig": wbig, "xstem": xstem[i].reshape(128, -1), "fcl": fcl,
                "sel": sel} for i in range(CORES)]
    res = run_bass_kernel_spmd(nc, in_maps, list(range(CORES)))
    out = np.concatenate([r["out"].T for r in res.results], axis=0)
    return out.astype(np.float32)


# revision 8
# speedup vs baseline: 1.0932x; 1.0932x over previous
"""AdderNet on 8 TRN2 cores — low-instruction-count architecture.

Per conv: activations live in DRAM as img [ci, B, H, W] f16. Build a
replicated per-partition window buffer Yf via broadcast DMAs (partition
p = co*lhn+lh holds the rows its output slice needs), then 9 in-place
tensor_tensor subtracts (one per kernel offset s) into D[128, s, l', ci],
tensor_reduce(|.|, negate) over ci, tensor_reduce over s -> R = -sum|x-w|.
BN stats via accum_out + AllReduce; evac with ACT bias/scale.
"""
import numpy as np

CORES = 8
BL = 4
GB = 32
EPS = 1e-5

# stage geometries keyed by out-geom
GEOM = {
    "X": dict(cn=16, lhn=8, lp=512, H=32),
    "Y": dict(cn=32, lhn=4, lp=256, H=16),
    "Z": dict(cn=64, lhn=2, lp=128, H=8),
}


def make_sched():
    cv = []

    def add(name, gin, gout, ci, k, st, wsrc, evac, idn=None, yin=None, yout=None):
        cv.append(dict(name=name, gin=gin, gout=gout, ci=ci, k=k, st=st,
                       wsrc=wsrc, evac=evac, idn=idn, yin=yin, yout=yout))

    add("stem", "S", "X", 3, 3, 1, ("conv1_w",), "relu", yout=0)
    rot = [(0, 1, 2), (2, 0, 1), (1, 2, 0)]
    for b in range(3):
        i, m, o = rot[b]
        add(f"l1b{b}c1", "X", "X", 16, 3, 1, ("l1_w", 2 * b), "relu", yin=i, yout=m)
        add(f"l1b{b}c2", "X", "X", 16, 3, 1, ("l1_w", 2 * b + 1), "res", idn=i,
            yin=m, yout=o)
    add("l2tc1", "TX", "Y", 16, 3, 2, ("l2_w0",), "relu", yin=1, yout=0)
    add("l2td", "TX", "Y", 16, 1, 2, ("l2_down",), "iden", yin=1, yout=3)
    add("l2tc2", "Y", "Y", 32, 3, 1, ("l2_ws", 0), "res", idn=3, yin=0, yout=1)
    rot2 = [(1, 2, 0), (0, 2, 1)]
    for b in range(2):
        i, m, o = rot2[b]
        add(f"l2b{b}c1", "Y", "Y", 32, 3, 1, ("l2_ws", 1 + 2 * b), "relu", yin=i, yout=m)
        add(f"l2b{b}c2", "Y", "Y", 32, 3, 1, ("l2_ws", 2 + 2 * b), "res", idn=i,
            yin=m, yout=o)
    add("l3tc1", "TY", "Z", 32, 3, 2, ("l3_w0",), "relu", yin=1, yout=0)
    add("l3td", "TY", "Z", 32, 1, 2, ("l3_down",), "iden", yin=1, yout=3)
    add("l3tc2", "Z", "Z", 64, 3, 1, ("l3_ws", 0), "res", idn=3, yin=0, yout=1)
    for b in range(2):
        i, m, o = rot2[b]
        add(f"l3b{b}c1", "Z", "Z", 64, 3, 1, ("l3_ws", 1 + 2 * b), "relu", yin=i, yout=m)
        add(f"l3b{b}c2", "Z", "Z", 64, 3, 1, ("l3_ws", 2 + 2 * b), "res", idn=i,
            yin=m, yout=o)
    return cv


SCHED = make_sched()


def woffsets():
    offs = {}
    o = 0
    for m in SCHED:
        offs[m["name"]] = o
        o += m["k"] * m["k"] * m["ci"]
    return offs, o


WOFF, WTOT = woffsets()


def get_w(inputs, wsrc):
    a = inputs[wsrc[0]]
    if len(wsrc) > 1:
        a = a[wsrc[1]]
    return a


def pack_host(inputs):
    wbig = np.zeros((128, WTOT), np.float16)
    for meta in SCHED:
        w = get_w(inputs, meta["wsrc"])  # [co, ci, k, k]
        g = GEOM[meta["gout"]]
        cn, lhn = g["cn"], g["lhn"]
        k, ci = meta["k"], meta["ci"]
        off = WOFF[meta["name"]]
        wf = w.reshape(cn, ci, k * k).transpose(0, 2, 1).reshape(cn, k * k * ci)
        for p in range(128):
            wbig[p, off:off + k * k * ci] = wf[p // lhn]
    # stem input: [128, 3, 18, 34] per core
    x = inputs["x"].astype(np.float16)  # [32, 3, 32, 32]
    xs = x.reshape(CORES, BL, 3, 32, 32)
    xstem = np.zeros((CORES, 128, 3, 18, 34), np.float16)
    xpad = np.zeros((CORES, BL, 3, 34, 34), np.float16)
    xpad[:, :, :, 1:33, 1:33] = xs
    for p in range(128):
        lh = p % 8
        b, rh = divmod(lh, 2)
        xstem[:, p] = xpad[:, b, :, rh * 16:rh * 16 + 18, :]
    # fc lhsT [128, 20] f32: cols lh*10..+10 = fcw[cls, co]/64 on partitions with
    # p % lhn == lh... p=(co,lh): lh = p % 2, co = p // 2
    fcl = np.zeros((128, 20), np.float32)
    fcw = inputs["fc_w"][:, :, 0, 0]  # [10, 64]
    for p in range(128):
        co, lh = p // 2, p % 2
        fcl[p, lh * 10:(lh + 1) * 10] = fcw[:, co] / 64.0
    sel = np.zeros((64, 384), np.float32)
    for j, lhn in enumerate((8, 4, 2)):
        for p in range(128):
            sel[p // lhn, j * 128 + p] = 1.0
    return wbig, xstem, fcl, sel


_CACHE = {}


def build(debug=False, reps=1):
    from concourse import bacc, mybir, tile

    F16, F32 = mybir.dt.float16, mybir.dt.float32
    A = mybir.AluOpType
    AF = mybir.ActivationFunctionType
    AX = mybir.AxisListType

    nc = bacc.Bacc("TRN2", target_bir_lowering=False, debug=False,
                   num_devices=CORES)
    wbig_d = nc.dram_tensor("wbig", [128, WTOT], F16, kind="ExternalInput")
    xstem_d = nc.dram_tensor("xstem", [128, 3 * 18 * 34], F16, kind="ExternalInput")
    fcl_d = nc.dram_tensor("fcl", [128, 20], F32, kind="ExternalInput")
    sel_d = nc.dram_tensor("sel", [64, 384], F32, kind="ExternalInput")
    out_d = nc.dram_tensor("out", [10, BL], F32, kind="ExternalOutput")
    dbg_d = {}
    if debug:
        for meta in SCHED:
            g = GEOM[meta["gout"]]
            dbg_d[meta["name"]] = nc.dram_tensor(
                f'dbg_{meta["name"]}', [128, g["lp"]], F16, kind="ExternalOutput")

    with tile.TileContext(nc) as tc:
        import contextlib
        with contextlib.ExitStack() as ctx:
            pp = ctx.enter_context(tc.tile_pool(name="persist", bufs=1))
            psp = ctx.enter_context(tc.tile_pool(name="psum", bufs=1, space="PSUM"))
            drp = ctx.enter_context(tc.tile_pool(name="dram", bufs=1, space="DRAM"))

            wst = pp.tile([128, 3200], F16, tag="wst")
            gbase = [0]  # current group's base WOFF
            fcl = pp.tile([128, 20], F32, tag="fcl")
            sel = pp.tile([64, 384], F32, tag="sel")
            nc.sync.dma_start(fcl[:], fcl_d[:])
            nc.sync.dma_start(sel[:], sel_d[:])
            epst = pp.tile([64, 1], F32, tag="epst")
            nc.vector.memset(epst[:], EPS)

            D = pp.tile([128, 73728], F16, tag="D")
            nc.vector.memset(D[:, 0:1024], 0.0)
            Yf = pp.tile([128, 20736], F16, tag="Yf")
            o1 = pp.tile([128, 1296], F16, tag="o1")
            R = pp.tile([128, 512], F32, tag="R")
            junk = D[:, 0:512]
            tres = D[:, 512:1024]
            yn = [pp.tile([128, 512], F16, tag=f"yn{i}", name=f"yn{i}")
                  for i in range(4)]
            stats = pp.tile([128, 4], F32, tag="stats")
            gst = pp.tile([64, 8, 4], F32, tag="gst")
            g2 = pp.tile([64, 4], F32, tag="g2")
            bnm = pp.tile([64, 4], F32, tag="bnm")
            rr = pp.tile([64, 4], F32, tag="rr")
            rrb = pp.tile([128, 2], F32, tag="rrb")
            R2 = pp.tile([128, 512], F32, tag="R2")
            pair_R = [None]

            imgX = drp.tile([16, 4, 32, 34], F16, tag="imgX")
            imgY = drp.tile([32, 4, 16, 18], F16, tag="imgY")
            imgZ = drp.tile([64, 4, 8, 10], F16, tag="imgZ")
            sin = drp.tile([128, 4], F32, tag="sin")
            sout = drp.tile([128, 4], F32, tag="sout")
            sinf = drp.tile([10, 2], F32, tag="sinf")
            soutf = drp.tile([10, 2], F32, tag="soutf")
            IMG = {"X": imgX, "Y": imgY, "Z": imgZ}
            nc.sync.dma_start(
                imgX.rearrange("c b (rh r) w -> (c b rh) (r w)", rh=2),
                D[:, 0:544])
            nc.sync.dma_start(
                imgY.rearrange("c b h w -> (c b) (h w)"), D[:, 0:288])
            nc.sync.dma_start(
                imgZ.rearrange("c (bh b2) h w -> (c bh) (b2 h w)", b2=2),
                D[:, 0:160])


            # Yf views per input-geometry
            def yf_view(gin):
                if gin == "S":
                    return Yf[:, :3 * 18 * 34].rearrange(
                        "p (c h w) -> p c h w", c=3, h=18)
                if gin == "X":
                    return Yf[:, :16 * 18 * 34].rearrange(
                        "p (c h w) -> p c h w", c=16, h=18)
                if gin == "Y":
                    return Yf[:, :32 * 18 * 18].rearrange(
                        "p (c h w) -> p c h w", c=32, h=18)
                if gin == "Z":
                    return Yf[:, :64 * 20 * 10].rearrange(
                        "p (c h w) -> p c h w", c=64, h=20)
                if gin == "TX":
                    return Yf[:, :16 * 34 * 34].rearrange(
                        "p (c h w) -> p c h w", c=16, h=34)
                if gin == "TY":
                    return Yf[:, :32 * 36 * 18].rearrange(
                        "p (c h w) -> p c h w", c=32, h=36)

            def build_yf(meta):
                """Regather input img -> Yf (replicated windows)."""
                gin = meta["gin"]
                go = GEOM[meta["gout"]]
                yv = yf_view(gin)
                if gin == "S":
                    nc.sync.dma_start(Yf[:, :3 * 18 * 34], xstem_d[:])
                    return
                if gin == "X":  # 18-row slices per lh, full 34-wide rows
                    for lh in range(8):
                        b, rh = divmod(lh, 2)
                        r0 = rh * 16 - 1
                        r1 = rh * 16 + 17
                        c0, c1 = max(r0, 0), min(r1, 32)
                        dr0 = c0 - r0
                        src = imgX[None, :, b, c0:c1, :].broadcast_to(
                            [16, 16, c1 - c0, 34]).rearrange(
                            "q c r w -> q c (r w)")
                        dst = yv[lh::8, :, dr0:dr0 + c1 - c0, :].rearrange(
                            "p c r w -> p c (r w)")
                        nc.sync.dma_start(dst, src)
                elif gin == "Y":
                    for lh in range(4):
                        src = imgY[None, :, lh, :, :].broadcast_to(
                            [32, 32, 16, 18]).rearrange("q c r w -> q c (r w)")
                        dst = yv[lh::4, :, 1:17, :].rearrange(
                            "p c r w -> p c (r w)")
                        nc.sync.dma_start(dst, src)
                elif gin == "Z":
                    for lh in range(2):
                        for b2 in range(2):
                            src = imgZ[None, :, 2 * lh + b2, :, :].broadcast_to(
                                [64, 64, 8, 10]).rearrange(
                                "q c r w -> q c (r w)")
                            dst = yv[lh::2, :, b2 * 10 + 1:b2 * 10 + 9,
                                     :].rearrange("p c r w -> p c (r w)")
                            nc.sync.dma_start(dst, src)
                elif gin == "TX":
                    for lh in range(4):
                        src = imgX[None, :, lh, :, :].broadcast_to(
                            [32, 16, 32, 34]).rearrange("q c r w -> q c (r w)")
                        dst = yv[lh::4, :, 1:33, :].rearrange(
                            "p c r w -> p c (r w)")
                        nc.sync.dma_start(dst, src)
                elif gin == "TY":
                    for lh in range(2):
                        for b2 in range(2):
                            src = imgY[None, :, 2 * lh + b2, :, :].broadcast_to(
                                [64, 32, 16, 18]).rearrange(
                                "q c r w -> q c (r w)")
                            dst = yv[lh::2, :, b2 * 18 + 1:b2 * 18 + 17,
                                     :].rearrange("p c r w -> p c (r w)")
                            nc.sync.dma_start(dst, src)

            def conv_D(meta):
                """tt subtracts + treds -> R[:, :lp] = -sum|x-w| (or conv)."""
                gin, gout = meta["gin"], meta["gout"]
                go = GEOM[gout]
                ci, k, st = meta["ci"], meta["k"], meta["st"]
                if meta["name"] in ("stem", "l2tc1", "l3tc1"):
                    gbase[0] = WOFF[meta["name"]]
                    gend = (WTOT if meta["name"] == "l3tc1"
                            else WOFF["l2tc1"] if meta["name"] == "stem"
                            else WOFF["l3tc1"])
                    nc.sync.dma_start(wst[:, :gend - gbase[0]],
                                      wbig_d[:, gbase[0]:gend])
                off = WOFF[meta["name"]] - gbase[0]
                wc = wst[:, off:off + k * k * ci]
                yv = yf_view(gin)
                is_stem = meta["name"] == "stem"
                op = A.mult if is_stem else A.subtract
                kk = k * k
                ABS = dict(apply_absolute_value=not is_stem,
                           negate=not is_stem)
                if gout in ("X", "Y"):
                    H = go["H"] if gout == "Y" else 16
                    W = 32 if gout == "X" else 16
                    dv = D[:, :H * W * kk * ci].rearrange(
                        "p (r w s c) -> p r w s c", r=H, w=W, s=kk)
                    for s in range(kk):
                        kh, kw = divmod(s, k)
                        if k == 1:
                            sv = yv[:, :, 1:33:2, 1:33:2]
                        elif st == 2:
                            sv = yv[:, :, kh:kh + 2 * H:2, kw:kw + 2 * W:2]
                        else:
                            sv = yv[:, :, kh:kh + H, kw:kw + W]
                        wv = wc[:, s * ci:(s + 1) * ci][
                            :, None, None, :].broadcast_to([128, H, W, ci])
                        nc.vector.tensor_tensor(dv[:, :, :, s, :], sv.transpose(
                            [0, 2, 3, 1]), wv, op)
                    dm = D[:, :H * W * kk * ci].rearrange(
                        "p (r w sc) -> p r w sc", r=H, w=W)
                    rv = R[:, :H * W].rearrange("p (r w) -> p r w", r=H)
                    nch = 2 if H * W * kk * ci > 65536 else 1
                    h = H // nch
                    for c in range(nch):
                        nc.vector.tensor_reduce(
                            rv[:, c * h:(c + 1) * h, :],
                            dm[:, c * h:(c + 1) * h, :, :],
                            AX.X, A.add, **ABS)
                else:  # gout Z: stacked pair [ci, 20, 10]
                    # out-row grid: stride1 -> 18 rows (im0 0..7, junk 8,9,
                    # im1 10..17); stride2 -> 17 rows (junk row 8 only)
                    G = 18 if st == 1 else 17
                    rb = 10 if st == 1 else 9
                    sgroups = [(0, 5), (5, kk)] if kk > 1 else [(0, 1)]
                    rv = R[:, :128].rearrange("p (b r w) -> p b r w", b=2, r=8)
                    for (s0, s1) in sgroups:
                        ns = s1 - s0
                        dv = D[:, :ns * G * 8 * ci].rearrange(
                            "p (s r w c) -> p s r w c", s=ns, r=G, w=8)
                        for s in range(s0, s1):
                            kh, kw = divmod(s, k)
                            if k == 1:
                                sv = yv[:, :, 1:1 + 2 * G:2, 1:17:2]
                            elif st == 2:
                                sv = yv[:, :, kh:kh + 2 * G:2, kw:kw + 16:2]
                            else:
                                sv = yv[:, :, kh:kh + G, kw:kw + 8]
                            wv = wc[:, s * ci:(s + 1) * ci][
                                :, None, None, :].broadcast_to([128, G, 8, ci])
                            nc.vector.tensor_tensor(
                                dv[:, s - s0], sv.transpose([0, 2, 3, 1]),
                                wv, A.subtract)
                        dm = D[:, :ns * G * 8 * ci].rearrange(
                            "p (sr w c) -> p sr w c", w=8, c=ci)
                        ov = o1[:, s0 * G * 8:s1 * G * 8].rearrange(
                            "p (s r w) -> p s r w", s=ns, r=G)
                        with nc.allow_low_precision(reason="f32 accum f16 out"):
                            nc.vector.tensor_reduce(
                                ov[:].rearrange("p s r w -> p (s r) w"),
                                dm[:], AX.X, A.add, **ABS)
                    o3 = o1[:, :kk * G * 8].rearrange(
                        "p (s r w) -> p s r w", s=kk, r=G)
                    for b2 in range(2):
                        r0 = b2 * rb
                        if kk == 1:
                            nc.vector.tensor_copy(
                                rv[:, b2], o3[:, 0, r0:r0 + 8, :])
                        else:
                            nc.vector.tensor_reduce(
                                rv[:, b2],
                                o3[:, :, r0:r0 + 8, :].transpose([0, 2, 3, 1]),
                                AX.X, A.add)

            def stats_op(meta, col):
                lp = GEOM[meta["gout"]]["lp"]
                nc.vector.tensor_scalar(junk[:, :lp], R[:, :lp], 0.0, None,
                                        A.add, A.add,
                                        accum_out=stats[:, col:col + 1])
                nc.scalar.activation(junk[:, :lp], R[:, :lp], AF.Square,
                                     accum_out=stats[:, col + 1:col + 2])

            def ar_bn(metas):
                """AllReduce stats cols [0:2*len(metas)] and fill rr pairs."""
                g = GEOM[metas[0]["gout"]]
                cn, lhn = g["cn"], g["lhn"]
                n = GB * g["H"] * g["H"]
                nv = len(metas)
                w = 2 * nv
                nc.sync.dma_start(sin[:, :w], stats[:, :w])
                nc.gpsimd.collective_compute(
                    "AllReduce", A.add, replica_groups=[list(range(CORES))],
                    ins=[sin.opt()], outs=[sout.opt()])
                nc.sync.dma_start(
                    gst[:cn, :lhn, :w],
                    sout[:, :w].rearrange("(c l) s -> c l s", c=cn))
                nc.vector.tensor_reduce(g2[:cn, :w], gst[:cn, :lhn, :w].transpose(
                    [0, 2, 1]), AX.X, A.add)
                s1 = g2[:cn, 0:w:2]
                s2 = g2[:cn, 1:w:2]
                mm = bnm[:cn, 0:w:2]
                vv = bnm[:cn, 1:w:2]
                if nv == 1:
                    nc.vector.tensor_scalar(mm, s1, 1.0 / n, None, A.mult)
                    nc.vector.scalar_tensor_tensor(vv, s1, mm, s2, A.mult,
                                                   A.subtract)
                    nc.scalar.activation(vv, vv, AF.Sqrt, bias=epst[:cn],
                                         scale=-1.0 / n)
                    nc.vector.reciprocal(rr[:cn, 0:w:2], vv)
                    nc.vector.tensor_scalar(rr[:cn, 1:w:2], mm, -1.0,
                                            rr[:cn, 0:w:2], A.mult, A.mult)
                else:  # mm holds -m; signs arranged for plain tensor_tensor
                    nc.vector.tensor_scalar(mm, s1, -1.0 / n, None, A.mult)
                    nc.vector.tensor_tensor(vv, s1, mm, A.mult)
                    nc.vector.tensor_tensor(vv, s2, vv, A.add)
                    nc.scalar.activation(vv, vv, AF.Sqrt, bias=epst[:cn],
                                         scale=1.0 / n)
                    nc.vector.reciprocal(rr[:cn, 0:w:2], vv)
                    nc.vector.tensor_tensor(rr[:cn, 1:w:2], mm,
                                            rr[:cn, 0:w:2], A.mult)

            def evac(meta, col):
                g = GEOM[meta["gout"]]
                cn, lhn, lp = g["cn"], g["lhn"], g["lp"]
                gsl = {8: 0, 4: 1, 2: 2}[lhn]
                rrp = psp.tile([128, 2], F32, tag="rrp", name="rrp")
                nc.tensor.matmul(rrp[:, :],
                                 sel[:cn, gsl * 128:(gsl + 1) * 128],
                                 rr[:cn, col:col + 2], start=True, stop=True)
                nc.vector.tensor_copy(rrb[:], rrp[:])
                ynout = yn[meta["yout"]]
                if meta["evac"] == "relu":
                    nc.scalar.activation(ynout[:, :lp], R[:, :lp], AF.Relu,
                                         bias=rrb[:, 1:2], scale=rrb[:, 0:1])
                elif meta["evac"] == "iden":
                    nc.scalar.activation(ynout[:, :lp], R[:, :lp], AF.Identity,
                                         bias=rrb[:, 1:2], scale=rrb[:, 0:1])
                else:  # res
                    idt = yn[meta["idn"]]
                    nc.vector.scalar_tensor_tensor(
                        tres[:, :lp], R[:, :lp], rrb[:, 0:1], idt[:, :lp],
                        A.mult, A.add)
                    nc.scalar.activation(ynout[:, :lp], tres[:, :lp], AF.Relu,
                                         bias=rrb[:, 1:2])
                if meta["evac"] != "iden":
                    img = IMG[meta["gout"]]
                    if meta["gout"] == "X":
                        dst = img.rearrange("c b (rh r) w -> (c b rh) r w",
                                            rh=2)[:, :, 1:33]
                    elif meta["gout"] == "Y":
                        dst = img.rearrange("c b h w -> (c b) h w")[:, :, 1:17]
                    else:
                        dst = img.rearrange("c (bh b2) h w -> (c bh) (b2 h) w",
                                            b2=2)[:, :, 1:9]
                    nc.sync.dma_start(dst, ynout[:, :lp])
                if debug:
                    nc.sync.dma_start(dbg_d[meta["name"]][:], ynout[:, :lp])

            def bn_evac(meta):
                stats_op(meta, 0)
                ar_bn([meta])
                evac(meta, 0)

            # zero Yf at geometry switches
            prev_gin = None
            for _rep in range(reps):
                for meta in SCHED:
                    if meta["gin"] != prev_gin:
                        nc.vector.memset(Yf[:], 0.0)
                        prev_gin = meta["gin"]
                        build_yf(meta)
                    elif meta["gin"] not in ("TX", "TY"):
                        build_yf(meta)
                    if meta["name"] in ("l2tc1", "l3tc1"):
                        pair_R[0] = None
                        with nc.named_scope(meta["name"]):
                            conv_D(meta)
                            stats_op(meta, 0)
                            lp = GEOM[meta["gout"]]["lp"]
                            nc.vector.tensor_copy(R2[:, :lp], R[:, :lp])
                            pair_R[0] = meta
                    elif meta["name"] in ("l2td", "l3td"):
                        with nc.named_scope(meta["name"]):
                            conv_D(meta)
                            stats_op(meta, 2)
                            ar_bn([pair_R[0], meta])
                            evac(meta, 2)  # td evac uses R (its own)
                            lp = GEOM[meta["gout"]]["lp"]
                            nc.vector.tensor_copy(R[:, :lp], R2[:, :lp])
                            evac(pair_R[0], 0)
                    else:
                        with nc.named_scope(meta["name"]):
                            conv_D(meta)
                            bn_evac(meta)

                # ---- fc ----
                with nc.named_scope("fc"):
                    ynf = yn[SCHED[-1]["yout"]]
                    pool = pp.tile([128, 2], F32, tag="pool")
                    yv = ynf[:, :128].rearrange("p (b f) -> p b f", b=2)
                    for b2 in range(2):
                        nc.scalar.activation(junk[:, :64], yv[:, b2], AF.Identity,
                                             accum_out=pool[:, b2:b2 + 1])
                    psfc = psp.tile([10, 4], F32, tag="psfc")
                    for lh in range(2):
                        nc.tensor.matmul(psfc[:, lh * 2:lh * 2 + 2],
                                         fcl[:, lh * 10:(lh + 1) * 10],
                                         pool[:], start=True, stop=True)
                    stf = pp.tile([10, 2], F32, tag="stf")
                    jf = pp.tile([10, 4], F16, tag="jf")
                    nc.scalar.activation(jf[:], psfc[:], AF.Identity,
                                         accum_out=stf[:, 0:1])
                    nc.scalar.activation(jf[:], psfc[:], AF.Square,
                                         accum_out=stf[:, 1:2])
                    nc.sync.dma_start(sinf[:], stf[:])
                    nc.gpsimd.collective_compute(
                        "AllReduce", A.add, replica_groups=[list(range(CORES))],
                        ins=[sinf.opt()], outs=[soutf.opt()])
                    gstf = pp.tile([10, 2], F32, tag="gstf")
                    nc.sync.dma_start(gstf[:], soutf[:])
                    nc.vector.tensor_scalar(bnm[:10, 0:1], gstf[:, 0:1],
                                            1.0 / GB, None, A.mult)
                    nc.vector.tensor_tensor(bnm[:10, 1:2], bnm[:10, 0:1],
                                            bnm[:10, 0:1], A.mult)
                    nc.vector.tensor_scalar(rr[:10, 0:1], gstf[:, 1:2],
                                            1.0 / GB, bnm[:10, 1:2], A.mult,
                                            A.subtract)
                    nc.scalar.activation(bnm[:10, 1:2], rr[:10, 0:1], AF.Sqrt,
                                         bias=epst[:10])
                    nc.vector.reciprocal(rr[:10, 0:1], bnm[:10, 1:2])
                    nc.vector.tensor_scalar(rr[:10, 1:2], bnm[:10, 0:1], -1.0,
                                            rr[:10, 0:1], A.mult, A.mult)
                    osb = pp.tile([10, 4], F32, tag="osb")
                    nc.scalar.activation(osb[:], psfc[:], AF.Identity,
                                         bias=rr[:10, 1:2], scale=rr[:10, 0:1])
                    nc.sync.dma_start(out_d[:], osb[:])
                prev_gin = None if reps > 1 else prev_gin

    nc.compile()
    return nc


def get_nc(debug=False, reps=1):
    key = f"nc{debug}_{reps}"
    if key not in _CACHE:
        _CACHE[key] = build(debug, reps)
    return _CACHE[key]


def kernel(**inputs):
    from concourse.bass_utils import run_bass_kernel_spmd

    wbig, xstem, fcl, sel = pack_host(inputs)
    nc = get_nc()
    in_maps = [{"wbig": wbig, "xstem": xstem[i].reshape(128, -1), "fcl": fcl,
                "sel": sel} for i in range(CORES)]
    res = run_bass_kernel_spmd(nc, in_maps, list(range(CORES)))
    out = np.concatenate([r["out"].T for r in res.results], axis=0)
    return out.astype(np.float32)


# revision 10
# speedup vs baseline: 1.6119x; 1.4745x over previous
"""AdderNet on 8 TRN2 cores — low-instruction-count architecture.

Per conv: activations live in DRAM as img [ci, B, H, W] f16. Build a
replicated per-partition window buffer Yf via broadcast DMAs (partition
p = co*lhn+lh holds the rows its output slice needs), then 9 in-place
tensor_tensor subtracts (one per kernel offset s) into D[128, s, l', ci],
tensor_reduce(|.|, negate) over ci, tensor_reduce over s -> R = -sum|x-w|.
BN stats via accum_out + AllReduce; evac with ACT bias/scale.
"""
import numpy as np

CORES = 8
BL = 4
GB = 32
EPS = 1e-5

# stage geometries keyed by out-geom
GEOM = {
    "X": dict(cn=16, lhn=8, lp=512, H=32),
    "Y": dict(cn=32, lhn=4, lp=256, H=16),
    "Z": dict(cn=64, lhn=2, lp=128, H=8),
}


def make_sched():
    cv = []

    def add(name, gin, gout, ci, k, st, wsrc, evac, idn=None, yin=None, yout=None):
        cv.append(dict(name=name, gin=gin, gout=gout, ci=ci, k=k, st=st,
                       wsrc=wsrc, evac=evac, idn=idn, yin=yin, yout=yout))

    add("stem", "S", "X", 3, 3, 1, ("conv1_w",), "relu", yout=0)
    rot = [(0, 1, 2), (2, 0, 1), (1, 2, 0)]
    for b in range(3):
        i, m, o = rot[b]
        add(f"l1b{b}c1", "X", "X", 16, 3, 1, ("l1_w", 2 * b), "relu", yin=i, yout=m)
        add(f"l1b{b}c2", "X", "X", 16, 3, 1, ("l1_w", 2 * b + 1), "res", idn=i,
            yin=m, yout=o)
    add("l2tc1", "TX", "Y", 16, 3, 2, ("l2_w0",), "relu", yin=1, yout=0)
    add("l2td", "TX", "Y", 16, 1, 2, ("l2_down",), "iden", yin=1, yout=3)
    add("l2tc2", "Y", "Y", 32, 3, 1, ("l2_ws", 0), "res", idn=3, yin=0, yout=1)
    rot2 = [(1, 2, 0), (0, 2, 1)]
    for b in range(2):
        i, m, o = rot2[b]
        add(f"l2b{b}c1", "Y", "Y", 32, 3, 1, ("l2_ws", 1 + 2 * b), "relu", yin=i, yout=m)
        add(f"l2b{b}c2", "Y", "Y", 32, 3, 1, ("l2_ws", 2 + 2 * b), "res", idn=i,
            yin=m, yout=o)
    add("l3tc1", "TY", "Z", 32, 3, 2, ("l3_w0",), "relu", yin=1, yout=0)
    add("l3td", "TY", "Z", 32, 1, 2, ("l3_down",), "iden", yin=1, yout=3)
    add("l3tc2", "Z", "Z", 64, 3, 1, ("l3_ws", 0), "res", idn=3, yin=0, yout=1)
    for b in range(2):
        i, m, o = rot2[b]
        add(f"l3b{b}c1", "Z", "Z", 64, 3, 1, ("l3_ws", 1 + 2 * b), "relu", yin=i, yout=m)
        add(f"l3b{b}c2", "Z", "Z", 64, 3, 1, ("l3_ws", 2 + 2 * b), "res", idn=i,
            yin=m, yout=o)
    return cv


SCHED = make_sched()


def woffsets():
    offs = {}
    o = 0
    for m in SCHED:
        offs[m["name"]] = o
        o += m["k"] * m["k"] * m["ci"]
    return offs, o


WOFF, WTOT = woffsets()


def get_w(inputs, wsrc):
    a = inputs[wsrc[0]]
    if len(wsrc) > 1:
        a = a[wsrc[1]]
    return a


def pack_host(inputs):
    wbig = np.zeros((128, WTOT), np.float16)
    for meta in SCHED:
        w = get_w(inputs, meta["wsrc"])  # [co, ci, k, k]
        g = GEOM[meta["gout"]]
        cn, lhn = g["cn"], g["lhn"]
        k, ci = meta["k"], meta["ci"]
        off = WOFF[meta["name"]]
        wf = w.reshape(cn, ci, k * k).transpose(0, 2, 1).reshape(cn, k * k * ci)
        for p in range(128):
            wbig[p, off:off + k * k * ci] = wf[p // lhn]
    # stem input: [128, 3, 18, 34] per core
    x = inputs["x"].astype(np.float16)  # [32, 3, 32, 32]
    xs = x.reshape(CORES, BL, 3, 32, 32)
    xstem = np.zeros((CORES, 128, 3, 18, 34), np.float16)
    xpad = np.zeros((CORES, BL, 3, 34, 34), np.float16)
    xpad[:, :, :, 1:33, 1:33] = xs
    for p in range(128):
        lh = p % 8
        b, rh = divmod(lh, 2)
        xstem[:, p] = xpad[:, b, :, rh * 16:rh * 16 + 18, :]
    # fc lhsT [128, 20] f32: cols lh*10..+10 = fcw[cls, co]/64 on partitions with
    # p % lhn == lh... p=(co,lh): lh = p % 2, co = p // 2
    fcl = np.zeros((128, 20), np.float32)
    fcw = inputs["fc_w"][:, :, 0, 0]  # [10, 64]
    for p in range(128):
        co, lh = p // 2, p % 2
        fcl[p, lh * 10:(lh + 1) * 10] = fcw[:, co] / 64.0
    sel = np.zeros((64, 384), np.float32)
    for j, lhn in enumerate((8, 4, 2)):
        for p in range(128):
            sel[p // lhn, j * 128 + p] = 1.0
    selT = np.zeros((128, 112), np.float32)
    for base, lhn in ((0, 8), (16, 4), (48, 2)):
        for p in range(128):
            selT[p, base + p // lhn] = 1.0
    return wbig, xstem, fcl, sel, selT


_CACHE = {}


def build(debug=False, reps=1):
    from concourse import bacc, mybir, tile

    F16, F32 = mybir.dt.float16, mybir.dt.float32
    A = mybir.AluOpType
    AF = mybir.ActivationFunctionType
    AX = mybir.AxisListType

    nc = bacc.Bacc("TRN2", target_bir_lowering=False, debug=False,
                   num_devices=CORES)
    wbig_d = nc.dram_tensor("wbig", [128, WTOT], F16, kind="ExternalInput")
    xstem_d = nc.dram_tensor("xstem", [128, 3 * 18 * 34], F16, kind="ExternalInput")
    fcl_d = nc.dram_tensor("fcl", [128, 20], F32, kind="ExternalInput")
    sel_d = nc.dram_tensor("sel", [64, 384], F32, kind="ExternalInput")
    selT_d = nc.dram_tensor("selT", [128, 112], F32, kind="ExternalInput")
    out_d = nc.dram_tensor("out", [10, BL], F32, kind="ExternalOutput")
    dbg_d = {}
    if debug:
        for meta in SCHED:
            g = GEOM[meta["gout"]]
            dbg_d[meta["name"]] = nc.dram_tensor(
                f'dbg_{meta["name"]}', [128, g["lp"]], F16, kind="ExternalOutput")

    with tile.TileContext(nc) as tc:
        import contextlib
        with contextlib.ExitStack() as ctx:
            pp = ctx.enter_context(tc.tile_pool(name="persist", bufs=1))
            psp = ctx.enter_context(tc.tile_pool(name="psum", bufs=1, space="PSUM"))
            drp = ctx.enter_context(tc.tile_pool(name="dram", bufs=1, space="DRAM"))

            wst = pp.tile([128, 3200], F16, tag="wst")
            gbase = [0]  # current group's base WOFF
            fcl = pp.tile([128, 20], F32, tag="fcl")
            sel = pp.tile([64, 384], F32, tag="sel")
            selT = pp.tile([128, 112], F32, tag="selT")
            gsb = pp.tile([128, 4], F32, tag="gsb")
            nc.sync.dma_start(fcl[:], fcl_d[:])
            nc.sync.dma_start(sel[:], sel_d[:])
            nc.sync.dma_start(selT[:], selT_d[:])
            epst = pp.tile([64, 1], F32, tag="epst")
            nc.vector.memset(epst[:], EPS)

            D = pp.tile([128, 73728], F16, tag="D")
            nc.vector.memset(D[:, 0:1024], 0.0)
            Yf = pp.tile([128, 20736], F16, tag="Yf")
            o1 = pp.tile([128, 1296], F16, tag="o1")
            R = pp.tile([128, 512], F32, tag="R")
            junk = D[:, 0:512]
            tres = D[:, 512:1024]
            yn = [pp.tile([128, 512], F16, tag=f"yn{i}", name=f"yn{i}")
                  for i in range(4)]
            stats = pp.tile([128, 4], F32, tag="stats")
            gst = pp.tile([64, 8, 4], F32, tag="gst")
            g2 = pp.tile([64, 4], F32, tag="g2")
            bnm = pp.tile([64, 4], F32, tag="bnm")
            rr = pp.tile([64, 4], F32, tag="rr")
            rrb = pp.tile([128, 2], F32, tag="rrb")
            R2 = pp.tile([128, 512], F32, tag="R2")
            pair_R = [None]

            imgX = drp.tile([16, 4, 32, 34], F16, tag="imgX")
            imgY = drp.tile([32, 4, 16, 18], F16, tag="imgY")
            imgZ = drp.tile([64, 4, 8, 10], F16, tag="imgZ")
            sin = drp.tile([128, 4], F32, tag="sin")
            sout = drp.tile([128, 4], F32, tag="sout")
            sinf = drp.tile([10, 2], F32, tag="sinf")
            soutf = drp.tile([10, 2], F32, tag="soutf")
            IMG = {"X": imgX, "Y": imgY, "Z": imgZ}
            nc.sync.dma_start(
                imgX.rearrange("c b (rh r) w -> (c b rh) (r w)", rh=2),
                D[:, 0:544])
            nc.sync.dma_start(
                imgY.rearrange("c b h w -> (c b) (h w)"), D[:, 0:288])
            nc.sync.dma_start(
                imgZ.rearrange("c (bh b2) h w -> (c bh) (b2 h w)", b2=2),
                D[:, 0:160])


            # Yf views per input-geometry
            def yf_view(gin):
                if gin == "S":
                    return Yf[:, :3 * 18 * 34].rearrange(
                        "p (c h w) -> p c h w", c=3, h=18)
                if gin == "X":
                    return Yf[:, :16 * 18 * 34].rearrange(
                        "p (c h w) -> p c h w", c=16, h=18)
                if gin == "Y":
                    return Yf[:, :32 * 18 * 18].rearrange(
                        "p (c h w) -> p c h w", c=32, h=18)
                if gin == "Z":
                    return Yf[:, :64 * 20 * 10].rearrange(
                        "p (c h w) -> p c h w", c=64, h=20)
                if gin == "TX":
                    return Yf[:, :16 * 34 * 34].rearrange(
                        "p (c h w) -> p c h w", c=16, h=34)
                if gin == "TY":
                    return Yf[:, :32 * 36 * 18].rearrange(
                        "p (c h w) -> p c h w", c=32, h=36)

            def build_yf(meta):
                """Regather input img -> Yf (replicated windows)."""
                gin = meta["gin"]
                go = GEOM[meta["gout"]]
                yv = yf_view(gin)
                if gin == "S":
                    nc.sync.dma_start(Yf[:, :3 * 18 * 34], xstem_d[:])
                    return
                if gin == "X":  # 18-row slices per lh, full 34-wide rows
                    for lh in range(8):
                        b, rh = divmod(lh, 2)
                        r0 = rh * 16 - 1
                        r1 = rh * 16 + 17
                        c0, c1 = max(r0, 0), min(r1, 32)
                        dr0 = c0 - r0
                        src = imgX[None, :, b, c0:c1, :].broadcast_to(
                            [16, 16, c1 - c0, 34]).rearrange(
                            "q c r w -> q c (r w)")
                        dst = yv[lh::8, :, dr0:dr0 + c1 - c0, :].rearrange(
                            "p c r w -> p c (r w)")
                        nc.sync.dma_start(dst, src)
                elif gin == "Y":
                    for lh in range(4):
                        src = imgY[None, :, lh, :, :].broadcast_to(
                            [32, 32, 16, 18]).rearrange("q c r w -> q c (r w)")
                        dst = yv[lh::4, :, 1:17, :].rearrange(
                            "p c r w -> p c (r w)")
                        nc.sync.dma_start(dst, src)
                elif gin == "Z":
                    for lh in range(2):
                        for b2 in range(2):
                            src = imgZ[None, :, 2 * lh + b2, :, :].broadcast_to(
                                [64, 64, 8, 10]).rearrange(
                                "q c r w -> q c (r w)")
                            dst = yv[lh::2, :, b2 * 10 + 1:b2 * 10 + 9,
                                     :].rearrange("p c r w -> p c (r w)")
                            nc.sync.dma_start(dst, src)
                elif gin == "TX":
                    for lh in range(4):
                        src = imgX[None, :, lh, :, :].broadcast_to(
                            [32, 16, 32, 34]).rearrange("q c r w -> q c (r w)")
                        dst = yv[lh::4, :, 1:33, :].rearrange(
                            "p c r w -> p c (r w)")
                        nc.sync.dma_start(dst, src)
                elif gin == "TY":
                    for lh in range(2):
                        for b2 in range(2):
                            src = imgY[None, :, 2 * lh + b2, :, :].broadcast_to(
                                [64, 32, 16, 18]).rearrange(
                                "q c r w -> q c (r w)")
                            dst = yv[lh::2, :, b2 * 18 + 1:b2 * 18 + 17,
                                     :].rearrange("p c r w -> p c (r w)")
                            nc.sync.dma_start(dst, src)

            def conv_D(meta):
                """tt subtracts + treds -> R[:, :lp] = -sum|x-w| (or conv)."""
                gin, gout = meta["gin"], meta["gout"]
                go = GEOM[gout]
                ci, k, st = meta["ci"], meta["k"], meta["st"]
                if meta["name"] in ("stem", "l2tc1", "l3tc1"):
                    gbase[0] = WOFF[meta["name"]]
                    gend = (WTOT if meta["name"] == "l3tc1"
                            else WOFF["l2tc1"] if meta["name"] == "stem"
                            else WOFF["l3tc1"])
                    nc.sync.dma_start(wst[:, :gend - gbase[0]],
                                      wbig_d[:, gbase[0]:gend])
                off = WOFF[meta["name"]] - gbase[0]
                wc = wst[:, off:off + k * k * ci]
                yv = yf_view(gin)
                is_stem = meta["name"] == "stem"
                op = A.mult if is_stem else A.subtract
                kk = k * k
                ABS = dict(apply_absolute_value=not is_stem,
                           negate=not is_stem)
                if gout in ("X", "Y"):
                    H = go["H"] if gout == "Y" else 16
                    W = 32 if gout == "X" else 16
                    dv = D[:, :H * W * kk * ci].rearrange(
                        "p (r w s c) -> p r w s c", r=H, w=W, s=kk)
                    for s in range(kk):
                        kh, kw = divmod(s, k)
                        if k == 1:
                            sv = yv[:, :, 1:33:2, 1:33:2]
                        elif st == 2:
                            sv = yv[:, :, kh:kh + 2 * H:2, kw:kw + 2 * W:2]
                        else:
                            sv = yv[:, :, kh:kh + H, kw:kw + W]
                        wv = wc[:, s * ci:(s + 1) * ci][
                            :, None, None, :].broadcast_to([128, H, W, ci])
                        nc.vector.tensor_tensor(dv[:, :, :, s, :], sv.transpose(
                            [0, 2, 3, 1]), wv, op)
                    dm = D[:, :H * W * kk * ci].rearrange(
                        "p (r w sc) -> p r w sc", r=H, w=W)
                    rv = R[:, :H * W].rearrange("p (r w) -> p r w", r=H)
                    nch = 2 if H * W * kk * ci > 65536 else 1
                    h = H // nch
                    for c in range(nch):
                        nc.vector.tensor_reduce(
                            rv[:, c * h:(c + 1) * h, :],
                            dm[:, c * h:(c + 1) * h, :, :],
                            AX.X, A.add, **ABS)
                else:  # gout Z: stacked pair [ci, 20, 10]
                    # out-row grid: stride1 -> 18 rows (im0 0..7, junk 8,9,
                    # im1 10..17); stride2 -> 17 rows (junk row 8 only)
                    G = 18 if st == 1 else 17
                    rb = 10 if st == 1 else 9
                    sgroups = [(0, 5), (5, kk)] if kk > 1 else [(0, 1)]
                    rv = R[:, :128].rearrange("p (b r w) -> p b r w", b=2, r=8)
                    for (s0, s1) in sgroups:
                        ns = s1 - s0
                        dv = D[:, :ns * G * 8 * ci].rearrange(
                            "p (s r w c) -> p s r w c", s=ns, r=G, w=8)
                        for s in range(s0, s1):
                            kh, kw = divmod(s, k)
                            if k == 1:
                                sv = yv[:, :, 1:1 + 2 * G:2, 1:17:2]
                            elif st == 2:
                                sv = yv[:, :, kh:kh + 2 * G:2, kw:kw + 16:2]
                            else:
                                sv = yv[:, :, kh:kh + G, kw:kw + 8]
                            wv = wc[:, s * ci:(s + 1) * ci][
                                :, None, None, :].broadcast_to([128, G, 8, ci])
                            nc.vector.tensor_tensor(
                                dv[:, s - s0], sv.transpose([0, 2, 3, 1]),
                                wv, A.subtract)
                        dm = D[:, :ns * G * 8 * ci].rearrange(
                            "p (sr w c) -> p sr w c", w=8, c=ci)
                        ov = o1[:, s0 * G * 8:s1 * G * 8].rearrange(
                            "p (s r w) -> p s r w", s=ns, r=G)
                        with nc.allow_low_precision(reason="f32 accum f16 out"):
                            nc.vector.tensor_reduce(
                                ov[:].rearrange("p s r w -> p (s r) w"),
                                dm[:], AX.X, A.add, **ABS)
                    o3 = o1[:, :kk * G * 8].rearrange(
                        "p (s r w) -> p s r w", s=kk, r=G)
                    for b2 in range(2):
                        r0 = b2 * rb
                        if kk == 1:
                            nc.vector.tensor_copy(
                                rv[:, b2], o3[:, 0, r0:r0 + 8, :])
                        else:
                            nc.vector.tensor_reduce(
                                rv[:, b2],
                                o3[:, :, r0:r0 + 8, :].transpose([0, 2, 3, 1]),
                                AX.X, A.add)

            def stats_op(meta, col):
                lp = GEOM[meta["gout"]]["lp"]
                nc.vector.tensor_scalar(junk[:, :lp], R[:, :lp], 0.0, None,
                                        A.add, A.add,
                                        accum_out=stats[:, col:col + 1])
                nc.scalar.activation(junk[:, :lp], R[:, :lp], AF.Square,
                                     accum_out=stats[:, col + 1:col + 2])

            def ar_bn(metas):
                """AllReduce stats cols [0:2*len(metas)] and fill rr pairs."""
                g = GEOM[metas[0]["gout"]]
                cn, lhn = g["cn"], g["lhn"]
                n = GB * g["H"] * g["H"]
                nv = len(metas)
                w = 2 * nv
                nc.sync.dma_start(sin[:, :w], stats[:, :w])
                nc.gpsimd.collective_compute(
                    "AllReduce", A.add, replica_groups=[list(range(CORES))],
                    ins=[sin.opt()], outs=[sout.opt()])
                nc.sync.dma_start(
                    gst[:cn, :lhn, :w],
                    sout[:, :w].rearrange("(c l) s -> c l s", c=cn))
                nc.vector.tensor_reduce(g2[:cn, :w], gst[:cn, :lhn, :w].transpose(
                    [0, 2, 1]), AX.X, A.add)
                s1 = g2[:cn, 0:w:2]
                s2 = g2[:cn, 1:w:2]
                mm = bnm[:cn, 0:w:2]
                vv = bnm[:cn, 1:w:2]
                if nv == 1:
                    nc.vector.tensor_scalar(mm, s1, 1.0 / n, None, A.mult)
                    nc.vector.scalar_tensor_tensor(vv, s1, mm, s2, A.mult,
                                                   A.subtract)
                    nc.scalar.activation(vv, vv, AF.Sqrt, bias=epst[:cn],
                                         scale=-1.0 / n)
                    nc.vector.reciprocal(rr[:cn, 0:w:2], vv)
                    nc.vector.tensor_scalar(rr[:cn, 1:w:2], mm, -1.0,
                                            rr[:cn, 0:w:2], A.mult, A.mult)
                else:  # mm holds -m; signs arranged for plain tensor_tensor
                    nc.vector.tensor_scalar(mm, s1, -1.0 / n, None, A.mult)
                    nc.vector.tensor_tensor(vv, s1, mm, A.mult)
                    nc.vector.tensor_tensor(vv, s2, vv, A.add)
                    nc.scalar.activation(vv, vv, AF.Sqrt, bias=epst[:cn],
                                         scale=1.0 / n)
                    nc.vector.reciprocal(rr[:cn, 0:w:2], vv)
                    nc.vector.tensor_tensor(rr[:cn, 1:w:2], mm,
                                            rr[:cn, 0:w:2], A.mult)

            def evac(meta, col, rsrc=None):
                g = GEOM[meta["gout"]]
                cn, lhn, lp = g["cn"], g["lhn"], g["lp"]
                gsl = {8: 0, 4: 1, 2: 2}[lhn]
                rrp = psp.tile([128, 2], F32, tag="rrp", name="rrp")
                nc.tensor.matmul(rrp[:, :],
                                 sel[:cn, gsl * 128:(gsl + 1) * 128],
                                 rr[:cn, col:col + 2], start=True, stop=True)
                nc.vector.tensor_copy(rrb[:], rrp[:])
                ynout = yn[meta["yout"]]
                Rv = rsrc if rsrc is not None else R
                if meta["evac"] == "relu":
                    nc.scalar.activation(ynout[:, :lp], Rv[:, :lp], AF.Relu,
                                         bias=rrb[:, 1:2], scale=rrb[:, 0:1])
                elif meta["evac"] == "iden":
                    nc.scalar.activation(ynout[:, :lp], Rv[:, :lp], AF.Identity,
                                         bias=rrb[:, 1:2], scale=rrb[:, 0:1])
                else:  # res
                    idt = yn[meta["idn"]]
                    nc.vector.scalar_tensor_tensor(
                        tres[:, :lp], Rv[:, :lp], rrb[:, 0:1], idt[:, :lp],
                        A.mult, A.add)
                    nc.scalar.activation(ynout[:, :lp], tres[:, :lp], AF.Relu,
                                         bias=rrb[:, 1:2])
                if meta["evac"] != "iden":
                    img = IMG[meta["gout"]]
                    if meta["gout"] == "X":
                        dst = img.rearrange("c b (rh r) w -> (c b rh) r w",
                                            rh=2)[:, :, 1:33]
                    elif meta["gout"] == "Y":
                        dst = img.rearrange("c b h w -> (c b) h w")[:, :, 1:17]
                    else:
                        dst = img.rearrange("c (bh b2) h w -> (c bh) (b2 h) w",
                                            b2=2)[:, :, 1:9]
                    nc.sync.dma_start(dst, ynout[:, :lp])
                if debug:
                    nc.sync.dma_start(dbg_d[meta["name"]][:], ynout[:, :lp])

            def bn_evac(meta):
                stats_op(meta, 0)
                ar_bn([meta])
                evac(meta, 0)

            # zero Yf at geometry switches
            prev_gin = None
            for _rep in range(reps):
                for meta in SCHED:
                    if meta["gin"] != prev_gin:
                        nc.vector.memset(Yf[:], 0.0)
                        prev_gin = meta["gin"]
                        build_yf(meta)
                    elif meta["gin"] not in ("TX", "TY"):
                        build_yf(meta)
                    if meta["name"] in ("l2tc1", "l3tc1"):
                        pair_R[0] = None
                        with nc.named_scope(meta["name"]):
                            conv_D(meta)
                            stats_op(meta, 0)
                            lp = GEOM[meta["gout"]]["lp"]
                            nc.vector.tensor_copy(R2[:, :lp], R[:, :lp])
                            pair_R[0] = meta
                    elif meta["name"] in ("l2td", "l3td"):
                        with nc.named_scope(meta["name"]):
                            conv_D(meta)
                            stats_op(meta, 2)
                            ar_bn([pair_R[0], meta])
                            evac(meta, 2)  # td evac uses R (its own)
                            evac(pair_R[0], 0, rsrc=R2)
                    else:
                        with nc.named_scope(meta["name"]):
                            conv_D(meta)
                            bn_evac(meta)

                # ---- fc ----
                with nc.named_scope("fc"):
                    ynf = yn[SCHED[-1]["yout"]]
                    pool = pp.tile([128, 2], F32, tag="pool")
                    yv = ynf[:, :128].rearrange("p (b f) -> p b f", b=2)
                    for b2 in range(2):
                        nc.scalar.activation(junk[:, :64], yv[:, b2], AF.Identity,
                                             accum_out=pool[:, b2:b2 + 1])
                    psfc = psp.tile([10, 4], F32, tag="psfc")
                    for lh in range(2):
                        nc.tensor.matmul(psfc[:, lh * 2:lh * 2 + 2],
                                         fcl[:, lh * 10:(lh + 1) * 10],
                                         pool[:], start=True, stop=True)
                    stf = pp.tile([10, 2], F32, tag="stf")
                    jf = pp.tile([10, 4], F16, tag="jf")
                    nc.scalar.activation(jf[:], psfc[:], AF.Identity,
                                         accum_out=stf[:, 0:1])
                    nc.scalar.activation(jf[:], psfc[:], AF.Square,
                                         accum_out=stf[:, 1:2])
                    nc.sync.dma_start(sinf[:], stf[:])
                    nc.gpsimd.collective_compute(
                        "AllReduce", A.add, replica_groups=[list(range(CORES))],
                        ins=[sinf.opt()], outs=[soutf.opt()])
                    gstf = pp.tile([10, 2], F32, tag="gstf")
                    nc.sync.dma_start(gstf[:], soutf[:])
                    nc.vector.tensor_scalar(bnm[:10, 0:1], gstf[:, 0:1],
                                            1.0 / GB, None, A.mult)
                    nc.vector.tensor_tensor(bnm[:10, 1:2], bnm[:10, 0:1],
                                            bnm[:10, 0:1], A.mult)
                    nc.vector.tensor_scalar(rr[:10, 0:1], gstf[:, 1:2],
                                            1.0 / GB, bnm[:10, 1:2], A.mult,
                                            A.subtract)
                    nc.scalar.activation(bnm[:10, 1:2], rr[:10, 0:1], AF.Sqrt,
                                         bias=epst[:10])
                    nc.vector.reciprocal(rr[:10, 0:1], bnm[:10, 1:2])
                    nc.vector.tensor_scalar(rr[:10, 1:2], bnm[:10, 0:1], -1.0,
                                            rr[:10, 0:1], A.mult, A.mult)
                    osb = pp.tile([10, 4], F32, tag="osb")
                    nc.scalar.activation(osb[:], psfc[:], AF.Identity,
                                         bias=rr[:10, 1:2], scale=rr[:10, 0:1])
                    nc.sync.dma_start(out_d[:], osb[:])
                prev_gin = None if reps > 1 else prev_gin

    nc.compile()
    return nc


def get_nc(debug=False, reps=1):
    key = f"nc{debug}_{reps}"
    if key not in _CACHE:
        _CACHE[key] = build(debug, reps)
    return _CACHE[key]


def kernel(**inputs):
    from concourse.bass_utils import run_bass_kernel_spmd

    wbig, xstem, fcl, sel, selT = pack_host(inputs)
    nc = get_nc()
    in_maps = [{"wbig": wbig, "xstem": xstem[i].reshape(128, -1), "fcl": fcl,
                "sel": sel, "selT": selT} for i in range(CORES)]
    res = run_bass_kernel_spmd(nc, in_maps, list(range(CORES)))
    out = np.concatenate([r["out"].T for r in res.results], axis=0)
    return out.astype(np.float32)
